# revision 50
# baseline (speedup 1.0000x reference)
"""Trainium2 Bass kernel for the fuzzy joint-membership layer.

Math (derived from the reference 2-qubit circuit, verified vs oracle):
  out[b, 2p,   c] = 0.5 + 0.5*cos(theta_c)*cos(x0) - 0.5*sin(theta_c)*sin(x0)*sin(x1)
  out[b, 2p+1, c] = 0.5 + 0.5*cos(x0)*cos(x1)
where x0 = xf[b, pair_idx[b,p,0]], x1 = xf[b, pair_idx[b,p,1]].

Sharding: pure data parallel, batch 4096 -> 8 cores x 512 rows.

Gather strategy: host-inverted local_scatter (per-partition independent
indices, vectorized Q7 loop in local RAM) instead of ap_gather (whose Q7 loop
costs ~24ns/idx and serialized the old kernel at ~83us per call):
  - host builds A_main[b, pix] = output slot of the FIRST occurrence of pixel
    `pix` in row b's 920-entry list (slot j = pair j's x0, 460+j = x1), -1 if
    unused; and Acomb[b, slot] = slot of the NEXT occurrence of the same
    pixel (chained generations), -1 if none.
  - device: G0 = local_scatter(xh, A_main); Gk = local_scatter(G{k-1}, Acomb)
    resolves duplicate gathers generation by generation (entries of other
    generations harmlessly scatter zeros). Gk have disjoint support.
  - m = max index multiplicity per row (data-dependent, ~6), baked at build.
x is converted to f16 on host (payload for the 2-byte scatter; output err
~1e-3, tolerance 2e-2).

GPSIMD shares its SBUF port with the Vector engine, so DVE ops stall ~5-17x
while a scatter streams. Hence the merge sum_k Gk runs on the idle PE
(identity matmuls accumulating into PSUM, exact, free f16->f32) and the
even-output base runs on ACT; DVE only does the range reduction, W/E
products, and one scalar_tensor_tensor per class.

Device per 128-row tile:
  - DMA in xh [128,3072] f16, A_main [128,3072] i16, Acomb [128,920] i16
  - gpsimd: m local_scatters (1 x 3072-scan + (m-1) x 920-scan)
  - PE: psum[h] = sum_k I @ Gk[:, h*460:...], h=0 (x0s) / 1 (x1s)
  - DVE range-reduction (magic round) + ACT Sin per half:
      cv_h = cos(vals_h), svN_h = -sin(vals_h)
  - DVE: W = svN0*svN1 = sin*sin, E = cv0*cv1
  - per class c (all writes contiguous, class-major device layout):
      even block = ACT(cv0*hct_c + 0.5), += W*nhst_c on DVE
      odd block = ACT(E*0.5 + 0.5)
  - DMA out [128, 9200]; host permutes class-major -> [B, 920, 10].
"""

import math
import numpy as np

B, PIX, NPAIR, C = 4096, 3072, 460, 10
NG = 2 * NPAIR          # 920 gathered values per row
OUTW = NG * C           # 9200
NCORES = 8
BS = B // NCORES        # 512 rows per core
TILES = BS // 128       # 4

_cache = {}


def _ensure_path():
    try:
        import concourse  # noqa: F401
    except ImportError:
        import sys
        sys.path.insert(0, "/opt/trn_rl_repo")


def _npasses(m):
    """Copy passes: A1 fills occ 1 from G0; K_k fills occ {2k, 2k+1} from
    the single array holding its source pair {2j, 2j+1} (M1 = G0+G1 for
    j=0, else K_j). One DVE merge (M1) total, everything else chains."""
    return max(1, (m - 1) // 2 + 1) if m >= 2 else 1


def build_nc(m, bs=BS):
    """m = max per-row index multiplicity -> 1 main scatter + ceil(log2(m)) copy passes."""
    _ensure_path()
    from contextlib import ExitStack
    import concourse.tile as tile
    from concourse import bacc, mybir

    f32, f16, i16 = mybir.dt.float32, mybir.dt.float16, mybir.dt.int16
    Sin = mybir.ActivationFunctionType.Sin
    Copy = mybir.ActivationFunctionType.Copy
    Abs = mybir.ActivationFunctionType.Abs
    mult = mybir.AluOpType.mult
    add = mybir.AluOpType.add
    sub_ = mybir.AluOpType.subtract
    maxop = mybir.AluOpType.max
    ntiles = bs // 128

    P = _npasses(m)
    OUTW_DEV = C * NPAIR + NPAIR  # evens class-major + one odd base block
    nc = bacc.Bacc("TRN2", target_bir_lowering=False, debug=False)
    xh_ext = nc.declare_dram_parameter("xh", [bs, PIX], f16, isOutput=False)
    am_ext = nc.declare_dram_parameter("amain", [bs, PIX], i16, isOutput=False)
    ac_ext = nc.declare_dram_parameter("acomb", [bs, P * NG], i16, isOutput=False)
    th_ext = nc.declare_dram_parameter("theta", [128, C], f32, isOutput=False)
    id_ext = nc.declare_dram_parameter("idmat", [128, 128], f16, isOutput=False)
    out_ext = nc.declare_dram_parameter("out", [bs, OUTW_DEV], f16, isOutput=True)

    PI, TWO_PI = math.pi, 2 * math.pi
    MAGIC, INV2PI = 1.5 * 2 ** 23, 1.0 / (2 * math.pi)

    with tile.TileContext(nc) as tc, ExitStack() as ctx:
        cpool = ctx.enter_context(tc.tile_pool(name="const", bufs=1))
        pihalf = cpool.tile([128, 1], f32)
        nc.vector.memset(pihalf[:], PI / 2)
        zerob = cpool.tile([128, 1], f32)
        nc.vector.memset(zerob[:], 0.0)
        idmat = cpool.tile([128, 128], f16)
        nc.sync.dma_start(out=idmat[:], in_=id_ext[:, :])
        # ordering-barrier scratch: the scheduler likes to hoist tile t+1's
        # 11us main scan above tile t's last copy pass on the serial gpsimd
        # queue, delaying tile t's merge+compute. A 2-element no-op scatter
        # that reads tile t's last gen and writes into tile t+1's gens[0]
        # (WAW with the main scan) pins the order: gen5(t) < main(t+1).
        negidx = cpool.tile([128, 2], i16)
        nc.vector.memset(negidx[:], -1)

        xpool = ctx.enter_context(tc.tile_pool(name="xh", bufs=2))
        apool = ctx.enter_context(tc.tile_pool(name="amain", bufs=2))
        bpool = ctx.enter_context(tc.tile_pool(name="acomb", bufs=2))
        gpool = ctx.enter_context(tc.tile_pool(name="gens", bufs=3))
        ppool = ctx.enter_context(tc.tile_pool(name="psum", bufs=2, space="PSUM"))
        spool = ctx.enter_context(tc.tile_pool(name="trig", bufs=3))
        opool = ctx.enter_context(tc.tile_pool(name="ot", bufs=3))

        def trig(pool, src, width, tagp):
            """returns (cv, svN) = (cos(src), -sin(src)), width cols, f32.

            Range-reduce with round-to-nearest magic: n = (v/2pi + M) - M,
            -r = 2pi*n - v. Then -sin(v) = Sin(-r), cos(v) = Sin(pi/2 - |r|).
            """
            t1 = pool.tile([128, width], f32, tag=tagp + "t1")
            nc.vector.tensor_scalar(t1[:], src, INV2PI, MAGIC, mult, add)
            nc.vector.tensor_scalar(t1[:], t1[:], MAGIC, None, sub_)
            nc.vector.tensor_scalar(t1[:], t1[:], TWO_PI, None, mult)
            negr = pool.tile([128, width], f32, tag=tagp + "negr")
            nc.vector.tensor_tensor(negr[:], t1[:], src, sub_)
            nc.vector.tensor_scalar(t1[:], negr[:], -1.0, None, mult)
            nc.vector.tensor_tensor(t1[:], t1[:], negr[:], maxop)  # |r|
            cv = pool.tile([128, width], f32, tag=tagp + "cv")
            svN = pool.tile([128, width], f32, tag=tagp + "svN")
            nc.scalar.activation(svN[:], negr[:], Sin, bias=zerob[:, 0:1])
            nc.scalar.activation(cv[:], t1[:], Sin, bias=pihalf[:, 0:1], scale=-1.0)
            return cv, svN

        # theta coefficients: hct = 0.5*cos(theta), nhst = -0.5*sin(theta)
        th_sb = cpool.tile([128, C], f32)
        nc.sync.dma_start(out=th_sb[:], in_=th_ext[:, :])
        cvt, svNt = trig(cpool, th_sb[:], C, "th")
        hcoef = cpool.tile([128, 2 * C], f32)
        nc.vector.tensor_scalar(hcoef[:, 0:C], cvt[:], 0.5, None, mult)
        nc.vector.tensor_scalar(hcoef[:, C:2 * C], svNt[:], 0.5, None, mult)
        hct = hcoef[:, 0:C]        # 0.5*cos(theta)
        nhst = hcoef[:, C:2 * C]   # -0.5*sin(theta)


        state = {"prev_last_gen": None}

        def stage_a(t):
            """DMAs + scatter chain + M1 merge for tile t."""
            rows = slice(t * 128, (t + 1) * 128)
            xh = xpool.tile([128, PIX], f16, tag="xh", name="xh")
            nc.sync.dma_start(out=xh[:], in_=xh_ext[rows, :])
            am = apool.tile([128, PIX], i16, tag="am", name="am")
            nc.sync.dma_start(out=am[:], in_=am_ext[rows, :])
            ac = bpool.tile([128, P * NG], i16, tag="ac", name="ac")
            nc.sync.dma_start(out=ac[:], in_=ac_ext[rows, :])

            # scatter chain: G0 (main, occ 0), G1 (occ 1, from G0),
            # K_k (occ {2k, 2k+1}, from M1 if source pair is {0,1} else K_j).
            # Only ONE cross-engine merge (M1 = G0+G1 on DVE).
            g0 = gpool.tile([128, NG], f16, tag="g0", name="g0")
            if state["prev_last_gen"] is not None:
                nc.gpsimd.local_scatter(
                    g0[:, 0:2], state["prev_last_gen"][:, 0:2], negidx[:], 128, 2, 2
                )
            nc.gpsimd.local_scatter(g0[:], xh[:], am[:], 128, NG, PIX)
            pe_arrays = [g0]
            last_dst = g0
            if m >= 2:
                g1 = gpool.tile([128, NG], f16, tag="g1", name="g1")
                nc.gpsimd.local_scatter(g1[:], g0[:], ac[:, 0:NG], 128, NG, NG)
                pe_arrays = [g0, g1]
                last_dst = g1
                if m >= 3:
                    # M1 = G0+G1 via idle PE + ACT (DVE is saturated and
                    # its queue delays the scatter chain by ~4-7us)
                    m1 = gpool.tile([128, NG], f16, tag="m1", name="m1")
                    pm = [
                        ppool.tile([128, NPAIR], f32, tag=f"pm{h}", name=f"pm{h}")
                        for h in range(2)
                    ]
                    for k, g in enumerate((g0, g1)):
                        for h in range(2):
                            nc.tensor.matmul(
                                pm[h][:], idmat[:],
                                g[:, h * NPAIR:(h + 1) * NPAIR],
                                start=(k == 0), stop=(k == 1),
                            )
                    for h in range(2):
                        nc.scalar.activation(
                            m1[:, h * NPAIR:(h + 1) * NPAIR], pm[h][:], Copy
                        )
                    pe_arrays = [m1]
                    ks = {0: m1}
                    for kk in range(1, (m - 1) // 2 + 1):
                        srcj = kk - (1 << ((2 * kk).bit_length() - 2))
                        kt = gpool.tile([128, NG], f16, tag=f"k{kk}", name=f"k{kk}")
                        nc.gpsimd.local_scatter(
                            kt[:], ks[srcj][:], ac[:, kk * NG:(kk + 1) * NG],
                            128, NG, NG,
                        )
                        ks[kk] = kt
                        pe_arrays.append(kt)
                        last_dst = kt
            state["prev_last_gen"] = last_dst
            return rows, pe_arrays

        # Range-reduction: magic round on DVE (3 ops, with the *2pi and
        # -v fused into one scalar_tensor_tensor), |r| via ACT Abs:
        #   t1 = v/2pi + MAGIC; n = t1 - MAGIC (exact)
        #   negr = n*2pi - v; absr = Abs(negr)
        #   svN = Sin(negr) = -sin(v); cv = Sin(pi/2 - absr) = cos(v)
        def trig2(src_psum, tagp):
            t1 = spool.tile([128, NPAIR], f32, tag=tagp + "t1", name=tagp + "t1")
            nc.vector.tensor_scalar(t1[:], src_psum, INV2PI, MAGIC, mult, add)
            nc.vector.tensor_scalar(t1[:], t1[:], MAGIC, None, sub_)
            negr = spool.tile([128, NPAIR], f32, tag=tagp + "negr", name=tagp + "negr")
            nc.vector.scalar_tensor_tensor(negr[:], t1[:], TWO_PI, src_psum, mult, sub_)
            absr = spool.tile([128, NPAIR], f32, tag=tagp + "absr", name=tagp + "absr")
            nc.scalar.activation(absr[:], negr[:], Abs)
            cv = spool.tile([128, NPAIR], f32, tag=tagp + "cv", name=tagp + "cv")
            svN = spool.tile([128, NPAIR], f32, tag=tagp + "svN", name=tagp + "svN")
            nc.scalar.activation(svN[:], negr[:], Sin, bias=zerob[:, 0:1])
            nc.scalar.activation(cv[:], absr[:], Sin, bias=pihalf[:, 0:1], scale=-1.0)
            return cv, svN

        def stage_b(rows, pe_arrays):
            """PE merge + trig + class math + output DMAs for one tile."""
            ph = [
                ppool.tile([128, NPAIR], f32, tag=f"ph{h}", name=f"ph{h}")
                for h in range(2)
            ]
            for k, g in enumerate(pe_arrays):
                for h in range(2):
                    nc.tensor.matmul(
                        ph[h][:], idmat[:],
                        g[:, h * NPAIR:(h + 1) * NPAIR],
                        start=(k == 0), stop=(k == len(pe_arrays) - 1),
                    )

            cv0, svN0 = trig2(ph[0][:], "h0")
            cv1, svN1 = trig2(ph[1][:], "h1")
            w = spool.tile([128, NPAIR], f32, tag="w")
            e = spool.tile([128, NPAIR], f32, tag="e")
            nc.vector.tensor_tensor(w[:], svN0[:], svN1[:], mult)
            nc.vector.tensor_tensor(e[:], cv0[:], cv1[:], mult)

            # parity-major, class-major device layout; host permutes back.
            # 5-class chunk tiles, each with its own DMA, so output streams
            # out while later chunks still compute (dep tracking is per-tile).
            HALF = C * NPAIR  # 4600
            CH = C // 2       # 5 classes per chunk
            CHW = CH * NPAIR  # 2300
            for ci in range(2):
                otc = opool.tile([128, CHW], f16, tag=f"ot_ev{ci}", name=f"ot_ev{ci}")
                for j in range(CH):
                    c = ci * CH + j
                    evs = otc[:, j * NPAIR: (j + 1) * NPAIR]
                    nc.scalar.activation(evs, cv0[:], Copy, bias=0.5, scale=hct[:, c:c + 1])
                    nc.vector.scalar_tensor_tensor(evs, w[:], nhst[:, c:c + 1], evs, mult, add)
                nc.sync.dma_start(
                    out=out_ext[rows, ci * CHW: (ci + 1) * CHW], in_=otc[:]
                )
            # odd outputs are class-independent: one base block; the host
            # broadcasts it across the 10 classes during unshard
            ov = opool.tile([128, NPAIR], f16, tag="ov", name="ov")
            nc.scalar.activation(ov[:], e[:], Copy, bias=0.5, scale=0.5)
            nc.sync.dma_start(out=out_ext[rows, HALF: HALF + NPAIR], in_=ov[:])

        # software-pipelined emission: tile t's scatters (and M1 merge) are
        # emitted BEFORE tile t-1's compute so the scheduler queues M1 ahead
        # of the previous tile's class ops on the DVE
        pending = None
        for t in range(ntiles):
            st = stage_a(t)
            if pending is not None:
                stage_b(*pending)
            pending = st
        stage_b(*pending)

    nc.compile()
    return nc


def _prep_indices(pair_idx):
    """Invert the per-row gather map into scatter index arrays.

    Slot layout: slot j = pair j's x0, slot 460+j = pair j's x1.
    Binary-doubling copy passes: occurrence o (o>=1) is filled in pass
    p = bit_length(o), sourced from occurrence o - 2^(p-1).
    Returns (A_main [B,3072] i16, Acopy [B,P*920] i16, m).
    """
    pi = np.asarray(pair_idx).reshape(B, NPAIR, 2)
    L = np.concatenate([pi[:, :, 0], pi[:, :, 1]], axis=1).astype(np.int64)
    order = np.argsort(L, axis=1, kind="stable")
    spix = np.take_along_axis(L, order, axis=1)
    first = np.ones_like(spix, dtype=bool)
    first[:, 1:] = spix[:, 1:] != spix[:, :-1]
    t = np.broadcast_to(np.arange(NG)[None, :], (B, NG))
    firstpos = np.maximum.accumulate(np.where(first, t, 0), axis=1)
    occ_sorted = t - firstpos
    m = int(occ_sorted.max()) + 1
    P = _npasses(m)

    A_main = np.full((B, PIX), -1, np.int16)
    rr, cc = np.nonzero(occ_sorted == 0)
    A_main[rr, spix[rr, cc]] = order[rr, cc].astype(np.int16)

    Acopy = np.full((B, P * NG), -1, np.int16)
    rr2, cc2 = np.nonzero(occ_sorted >= 1)
    o = occ_sorted[rr2, cc2]
    bl = np.floor(np.log2(o)).astype(np.int64) + 1         # bit_length(o)
    src_pos = firstpos[rr2, cc2] + (o - (1 << (bl - 1)))   # sorted pos of source occ
    src_slot = order[rr2, src_pos]
    # pass position: occ 1 -> array 0 (A1); occ {2k, 2k+1} -> array k (K_k)
    Acopy[rr2, (o // 2) * NG + src_slot] = order[rr2, cc2].astype(np.int16)
    return A_main, Acopy, m


def _idmat_np():
    if "idmat" not in _cache:
        _cache["idmat"] = np.ascontiguousarray(np.eye(128, dtype=np.float16))
    return _cache["idmat"]


def _get_nc(m):
    key = ("nc", m)
    if key not in _cache:
        _cache[key] = build_nc(m)
    return _cache[key]


def kernel(x, pair_idx, theta):
    _ensure_path()
    from concourse.bass_utils import run_bass_kernel_spmd

    xh = np.ascontiguousarray(
        np.asarray(x, dtype=np.float32).reshape(B, PIX).astype(np.float16)
    )
    A_main, Acomb, m = _prep_indices(pair_idx)
    thb = np.ascontiguousarray(
        np.tile(np.asarray(theta, dtype=np.float32).reshape(1, C), (128, 1))
    )
    nc = _get_nc(m)
    in_maps = [
        {
            "xh": xh[k * BS:(k + 1) * BS],
            "amain": A_main[k * BS:(k + 1) * BS],
            "acomb": Acomb[k * BS:(k + 1) * BS],
            "theta": thb,
            "idmat": _idmat_np(),
        }
        for k in range(NCORES)
    ]
    res = run_bass_kernel_spmd(nc, in_maps, list(range(NCORES))).results
    raw = np.concatenate([res[k]["out"] for k in range(NCORES)], axis=0)
    # device layout per row: [C, NPAIR] even outputs (class-major) + one
    # [NPAIR] odd base (odd outputs are class-independent); assemble
    # out[b, 2j + parity, c] with the odd base broadcast across classes
    HALF = C * NPAIR
    ev = raw[:, 0:HALF].astype(np.float32).reshape(B, C, NPAIR)
    od = raw[:, HALF: HALF + NPAIR].astype(np.float32)
    out = np.empty((B, NPAIR, 2, C), dtype=np.float32)
    out[:, :, 0, :] = ev.transpose(0, 2, 1)
    out[:, :, 1, :] = od[:, :, None]
    return out.reshape(B, NG, C)


# revision 51
# speedup vs baseline: 1.0162x; 1.0162x over previous
"""Trainium2 Bass kernel for the fuzzy joint-membership layer.

Math (derived from the reference 2-qubit circuit, verified vs oracle):
  out[b, 2p,   c] = 0.5 + 0.5*cos(theta_c)*cos(x0) - 0.5*sin(theta_c)*sin(x0)*sin(x1)
  out[b, 2p+1, c] = 0.5 + 0.5*cos(x0)*cos(x1)
where x0 = xf[b, pair_idx[b,p,0]], x1 = xf[b, pair_idx[b,p,1]].

Sharding: pure data parallel, batch 4096 -> 8 cores x 512 rows.

Gather strategy: host-inverted local_scatter (per-partition independent
indices, vectorized Q7 loop in local RAM) instead of ap_gather (whose Q7 loop
costs ~24ns/idx and serialized the old kernel at ~83us per call):
  - host builds A_main[b, pix] = output slot of the FIRST occurrence of pixel
    `pix` in row b's 920-entry list (slot j = pair j's x0, 460+j = x1), -1 if
    unused; and Acomb[b, slot] = slot of the NEXT occurrence of the same
    pixel (chained generations), -1 if none.
  - device: G0 = local_scatter(xh, A_main); Gk = local_scatter(G{k-1}, Acomb)
    resolves duplicate gathers generation by generation (entries of other
    generations harmlessly scatter zeros). Gk have disjoint support.
  - m = max index multiplicity per row (data-dependent, ~6), baked at build.
x is converted to f16 on host (payload for the 2-byte scatter; output err
~1e-3, tolerance 2e-2).

GPSIMD shares its SBUF port with the Vector engine, so DVE ops stall ~5-17x
while a scatter streams. Hence the merge sum_k Gk runs on the idle PE
(identity matmuls accumulating into PSUM, exact, free f16->f32) and the
even-output base runs on ACT; DVE only does the range reduction, W/E
products, and one scalar_tensor_tensor per class.

Device per 128-row tile:
  - DMA in xh [128,3072] f16, A_main [128,3072] i16, Acomb [128,920] i16
  - gpsimd: m local_scatters (1 x 3072-scan + (m-1) x 920-scan)
  - PE: psum[h] = sum_k I @ Gk[:, h*460:...], h=0 (x0s) / 1 (x1s)
  - DVE range-reduction (magic round) + ACT Sin per half:
      cv_h = cos(vals_h), svN_h = -sin(vals_h)
  - DVE: W = svN0*svN1 = sin*sin, E = cv0*cv1
  - per class c (all writes contiguous, class-major device layout):
      even block = ACT(cv0*hct_c + 0.5), += W*nhst_c on DVE
      odd block = ACT(E*0.5 + 0.5)
  - DMA out [128, 9200]; host permutes class-major -> [B, 920, 10].
"""

import math
import numpy as np

B, PIX, NPAIR, C = 4096, 3072, 460, 10
NG = 2 * NPAIR          # 920 gathered values per row
OUTW = NG * C           # 9200
NCORES = 8
BS = B // NCORES        # 512 rows per core
TILES = BS // 128       # 4

_cache = {}


def _ensure_path():
    try:
        import concourse  # noqa: F401
    except ImportError:
        import sys
        sys.path.insert(0, "/opt/trn_rl_repo")


def _npasses(m):
    """Copy passes: A1 fills occ 1 from G0; K_k fills occ {2k, 2k+1} from
    the single array holding its source pair {2j, 2j+1} (M1 = G0+G1 for
    j=0, else K_j). One DVE merge (M1) total, everything else chains."""
    return max(1, (m - 1) // 2 + 1) if m >= 2 else 1


def build_nc(m, bs=BS):
    """m = max per-row index multiplicity -> 1 main scatter + ceil(log2(m)) copy passes."""
    _ensure_path()
    from contextlib import ExitStack
    import concourse.tile as tile
    from concourse import bacc, mybir

    f32, f16, i16 = mybir.dt.float32, mybir.dt.float16, mybir.dt.int16
    Sin = mybir.ActivationFunctionType.Sin
    Copy = mybir.ActivationFunctionType.Copy
    Abs = mybir.ActivationFunctionType.Abs
    mult = mybir.AluOpType.mult
    add = mybir.AluOpType.add
    sub_ = mybir.AluOpType.subtract
    maxop = mybir.AluOpType.max
    ntiles = bs // 128

    P = _npasses(m)
    OUTW_DEV = C * NPAIR + NPAIR  # evens class-major + one odd base block
    nc = bacc.Bacc("TRN2", target_bir_lowering=False, debug=False)
    xh_ext = nc.declare_dram_parameter("xh", [bs, PIX], f16, isOutput=False)
    am_ext = nc.declare_dram_parameter("amain", [bs, PIX], i16, isOutput=False)
    ac_ext = nc.declare_dram_parameter("acomb", [bs, P * NG], i16, isOutput=False)
    th_ext = nc.declare_dram_parameter("theta", [128, C], f32, isOutput=False)
    id_ext = nc.declare_dram_parameter("idmat", [128, 128], f16, isOutput=False)
    out_ext = nc.declare_dram_parameter("out", [bs, OUTW_DEV], f16, isOutput=True)

    PI, TWO_PI = math.pi, 2 * math.pi
    MAGIC, INV2PI = 1.5 * 2 ** 23, 1.0 / (2 * math.pi)

    with tile.TileContext(nc) as tc, ExitStack() as ctx:
        cpool = ctx.enter_context(tc.tile_pool(name="const", bufs=1))
        pihalf = cpool.tile([128, 1], f32)
        nc.vector.memset(pihalf[:], PI / 2)
        zerob = cpool.tile([128, 1], f32)
        nc.vector.memset(zerob[:], 0.0)
        idmat = cpool.tile([128, 128], f16)
        nc.sync.dma_start(out=idmat[:], in_=id_ext[:, :])
        # ordering-barrier scratch: the scheduler likes to hoist tile t+1's
        # 11us main scan above tile t's last copy pass on the serial gpsimd
        # queue, delaying tile t's merge+compute. A 2-element no-op scatter
        # that reads tile t's last gen and writes into tile t+1's gens[0]
        # (WAW with the main scan) pins the order: gen5(t) < main(t+1).
        negidx = cpool.tile([128, 2], i16)
        nc.vector.memset(negidx[:], -1)

        xpool = ctx.enter_context(tc.tile_pool(name="xh", bufs=2))
        apool = ctx.enter_context(tc.tile_pool(name="amain", bufs=2))
        bpool = ctx.enter_context(tc.tile_pool(name="acomb", bufs=2))
        gpool = ctx.enter_context(tc.tile_pool(name="gens", bufs=3))
        ppool = ctx.enter_context(tc.tile_pool(name="psum", bufs=2, space="PSUM"))
        spool = ctx.enter_context(tc.tile_pool(name="trig", bufs=3))
        opool = ctx.enter_context(tc.tile_pool(name="ot", bufs=3))

        def trig(pool, src, width, tagp):
            """returns (cv, svN) = (cos(src), -sin(src)), width cols, f32.

            Range-reduce with round-to-nearest magic: n = (v/2pi + M) - M,
            -r = 2pi*n - v. Then -sin(v) = Sin(-r), cos(v) = Sin(pi/2 - |r|).
            """
            t1 = pool.tile([128, width], f32, tag=tagp + "t1")
            nc.vector.tensor_scalar(t1[:], src, INV2PI, MAGIC, mult, add)
            nc.vector.tensor_scalar(t1[:], t1[:], MAGIC, None, sub_)
            nc.vector.tensor_scalar(t1[:], t1[:], TWO_PI, None, mult)
            negr = pool.tile([128, width], f32, tag=tagp + "negr")
            nc.vector.tensor_tensor(negr[:], t1[:], src, sub_)
            nc.vector.tensor_scalar(t1[:], negr[:], -1.0, None, mult)
            nc.vector.tensor_tensor(t1[:], t1[:], negr[:], maxop)  # |r|
            cv = pool.tile([128, width], f32, tag=tagp + "cv")
            svN = pool.tile([128, width], f32, tag=tagp + "svN")
            nc.scalar.activation(svN[:], negr[:], Sin, bias=zerob[:, 0:1])
            nc.scalar.activation(cv[:], t1[:], Sin, bias=pihalf[:, 0:1], scale=-1.0)
            return cv, svN

        # theta coefficients: hct = 0.5*cos(theta), nhst = -0.5*sin(theta)
        th_sb = cpool.tile([128, C], f32)
        nc.sync.dma_start(out=th_sb[:], in_=th_ext[:, :])
        cvt, svNt = trig(cpool, th_sb[:], C, "th")
        hcoef = cpool.tile([128, 2 * C], f32)
        nc.vector.tensor_scalar(hcoef[:, 0:C], cvt[:], 0.5, None, mult)
        nc.vector.tensor_scalar(hcoef[:, C:2 * C], svNt[:], 0.5, None, mult)
        hct = hcoef[:, 0:C]        # 0.5*cos(theta)
        nhst = hcoef[:, C:2 * C]   # -0.5*sin(theta)


        state = {"prev_last_gen": None}

        def stage_a(t):
            """DMAs + scatter chain + M1 merge for tile t."""
            rows = slice(t * 128, (t + 1) * 128)
            xh = xpool.tile([128, PIX], f16, tag="xh", name="xh")
            nc.sync.dma_start(out=xh[:], in_=xh_ext[rows, :])
            am = apool.tile([128, PIX], i16, tag="am", name="am")
            nc.sync.dma_start(out=am[:], in_=am_ext[rows, :])
            ac = bpool.tile([128, P * NG], i16, tag="ac", name="ac")
            nc.sync.dma_start(out=ac[:], in_=ac_ext[rows, :])

            # scatter chain: G0 (main, occ 0), G1 (occ 1, from G0),
            # K_k (occ {2k, 2k+1}, from M1 if source pair is {0,1} else K_j).
            # Only ONE cross-engine merge (M1 = G0+G1 on DVE).
            g0 = gpool.tile([128, NG], f16, tag="g0", name="g0")
            if state["prev_last_gen"] is not None:
                nc.gpsimd.local_scatter(
                    g0[:, 0:2], state["prev_last_gen"][:, 0:2], negidx[:], 128, 2, 2
                )
            nc.gpsimd.local_scatter(g0[:], xh[:], am[:], 128, NG, PIX)
            pe_arrays = [g0]
            last_dst = g0
            if m >= 2:
                g1 = gpool.tile([128, NG], f16, tag="g1", name="g1")
                nc.gpsimd.local_scatter(g1[:], g0[:], ac[:, 0:NG], 128, NG, NG)
                pe_arrays = [g0, g1]
                last_dst = g1
                if m >= 3:
                    m1 = gpool.tile([128, NG], f16, tag="m1", name="m1")
                    nc.vector.tensor_tensor(m1[:], g0[:], g1[:], add)
                    pe_arrays = [m1]
                    ks = {0: m1}
                    for kk in range(1, (m - 1) // 2 + 1):
                        srcj = kk - (1 << ((2 * kk).bit_length() - 2))
                        kt = gpool.tile([128, NG], f16, tag=f"k{kk}", name=f"k{kk}")
                        nc.gpsimd.local_scatter(
                            kt[:], ks[srcj][:], ac[:, kk * NG:(kk + 1) * NG],
                            128, NG, NG,
                        )
                        ks[kk] = kt
                        pe_arrays.append(kt)
                        last_dst = kt
            state["prev_last_gen"] = last_dst
            return rows, pe_arrays

        # Range-reduction: magic round on DVE (3 ops, with the *2pi and
        # -v fused into one scalar_tensor_tensor), |r| via ACT Abs:
        #   t1 = v/2pi + MAGIC; n = t1 - MAGIC (exact)
        #   negr = n*2pi - v; absr = Abs(negr)
        #   svN = Sin(negr) = -sin(v); cv = Sin(pi/2 - absr) = cos(v)
        def trig2(src_psum, tagp):
            t1 = spool.tile([128, NPAIR], f32, tag=tagp + "t1", name=tagp + "t1")
            nc.vector.tensor_scalar(t1[:], src_psum, INV2PI, MAGIC, mult, add)
            nc.vector.tensor_scalar(t1[:], t1[:], MAGIC, None, sub_)
            negr = spool.tile([128, NPAIR], f32, tag=tagp + "negr", name=tagp + "negr")
            nc.vector.scalar_tensor_tensor(negr[:], t1[:], TWO_PI, src_psum, mult, sub_)
            absr = spool.tile([128, NPAIR], f32, tag=tagp + "absr", name=tagp + "absr")
            nc.scalar.activation(absr[:], negr[:], Abs)
            cv = spool.tile([128, NPAIR], f32, tag=tagp + "cv", name=tagp + "cv")
            svN = spool.tile([128, NPAIR], f32, tag=tagp + "svN", name=tagp + "svN")
            nc.scalar.activation(svN[:], negr[:], Sin, bias=zerob[:, 0:1])
            nc.scalar.activation(cv[:], absr[:], Sin, bias=pihalf[:, 0:1], scale=-1.0)
            return cv, svN

        def stage_b(rows, pe_arrays):
            """PE merge + trig + class math + output DMAs for one tile."""
            ph = [
                ppool.tile([128, NPAIR], f32, tag=f"ph{h}", name=f"ph{h}")
                for h in range(2)
            ]
            for k, g in enumerate(pe_arrays):
                for h in range(2):
                    nc.tensor.matmul(
                        ph[h][:], idmat[:],
                        g[:, h * NPAIR:(h + 1) * NPAIR],
                        start=(k == 0), stop=(k == len(pe_arrays) - 1),
                    )

            cv0, svN0 = trig2(ph[0][:], "h0")
            cv1, svN1 = trig2(ph[1][:], "h1")
            w = spool.tile([128, NPAIR], f32, tag="w")
            e = spool.tile([128, NPAIR], f32, tag="e")
            nc.vector.tensor_tensor(w[:], svN0[:], svN1[:], mult)
            nc.vector.tensor_tensor(e[:], cv0[:], cv1[:], mult)

            # parity-major, class-major device layout; host permutes back.
            # 5-class chunk tiles, each with its own DMA, so output streams
            # out while later chunks still compute (dep tracking is per-tile).
            HALF = C * NPAIR  # 4600
            CH = C // 2       # 5 classes per chunk
            CHW = CH * NPAIR  # 2300
            for ci in range(2):
                otc = opool.tile([128, CHW], f16, tag=f"ot_ev{ci}", name=f"ot_ev{ci}")
                for j in range(CH):
                    c = ci * CH + j
                    evs = otc[:, j * NPAIR: (j + 1) * NPAIR]
                    nc.scalar.activation(evs, cv0[:], Copy, bias=0.5, scale=hct[:, c:c + 1])
                    nc.vector.scalar_tensor_tensor(evs, w[:], nhst[:, c:c + 1], evs, mult, add)
                nc.sync.dma_start(
                    out=out_ext[rows, ci * CHW: (ci + 1) * CHW], in_=otc[:]
                )
            # odd outputs are class-independent: one base block; the host
            # broadcasts it across the 10 classes during unshard
            ov = opool.tile([128, NPAIR], f16, tag="ov", name="ov")
            nc.scalar.activation(ov[:], e[:], Copy, bias=0.5, scale=0.5)
            nc.sync.dma_start(out=out_ext[rows, HALF: HALF + NPAIR], in_=ov[:])

        # software-pipelined emission: tile t's scatters (and M1 merge) are
        # emitted BEFORE tile t-1's compute so the scheduler queues M1 ahead
        # of the previous tile's class ops on the DVE
        pending = None
        for t in range(ntiles):
            st = stage_a(t)
            if pending is not None:
                stage_b(*pending)
            pending = st
        stage_b(*pending)

    nc.compile()
    return nc


def _prep_indices(pair_idx):
    """Invert the per-row gather map into scatter index arrays.

    Slot layout: slot j = pair j's x0, slot 460+j = pair j's x1.
    Binary-doubling copy passes: occurrence o (o>=1) is filled in pass
    p = bit_length(o), sourced from occurrence o - 2^(p-1).
    Returns (A_main [B,3072] i16, Acopy [B,P*920] i16, m).
    """
    pi = np.asarray(pair_idx).reshape(B, NPAIR, 2)
    L = np.concatenate([pi[:, :, 0], pi[:, :, 1]], axis=1).astype(np.int64)
    order = np.argsort(L, axis=1, kind="stable")
    spix = np.take_along_axis(L, order, axis=1)
    first = np.ones_like(spix, dtype=bool)
    first[:, 1:] = spix[:, 1:] != spix[:, :-1]
    t = np.broadcast_to(np.arange(NG)[None, :], (B, NG))
    firstpos = np.maximum.accumulate(np.where(first, t, 0), axis=1)
    occ_sorted = t - firstpos
    m = int(occ_sorted.max()) + 1
    P = _npasses(m)

    A_main = np.full((B, PIX), -1, np.int16)
    rr, cc = np.nonzero(occ_sorted == 0)
    A_main[rr, spix[rr, cc]] = order[rr, cc].astype(np.int16)

    Acopy = np.full((B, P * NG), -1, np.int16)
    rr2, cc2 = np.nonzero(occ_sorted >= 1)
    o = occ_sorted[rr2, cc2]
    bl = np.floor(np.log2(o)).astype(np.int64) + 1         # bit_length(o)
    src_pos = firstpos[rr2, cc2] + (o - (1 << (bl - 1)))   # sorted pos of source occ
    src_slot = order[rr2, src_pos]
    # pass position: occ 1 -> array 0 (A1); occ {2k, 2k+1} -> array k (K_k)
    Acopy[rr2, (o // 2) * NG + src_slot] = order[rr2, cc2].astype(np.int16)
    return A_main, Acopy, m


def _idmat_np():
    if "idmat" not in _cache:
        _cache["idmat"] = np.ascontiguousarray(np.eye(128, dtype=np.float16))
    return _cache["idmat"]


def _get_nc(m):
    key = ("nc", m)
    if key not in _cache:
        _cache[key] = build_nc(m)
    return _cache[key]


def kernel(x, pair_idx, theta):
    _ensure_path()
    from concourse.bass_utils import run_bass_kernel_spmd

    xh = np.ascontiguousarray(
        np.asarray(x, dtype=np.float32).reshape(B, PIX).astype(np.float16)
    )
    A_main, Acomb, m = _prep_indices(pair_idx)
    thb = np.ascontiguousarray(
        np.tile(np.asarray(theta, dtype=np.float32).reshape(1, C), (128, 1))
    )
    nc = _get_nc(m)
    in_maps = [
        {
            "xh": xh[k * BS:(k + 1) * BS],
            "amain": A_main[k * BS:(k + 1) * BS],
            "acomb": Acomb[k * BS:(k + 1) * BS],
            "theta": thb,
            "idmat": _idmat_np(),
        }
        for k in range(NCORES)
    ]
    res = run_bass_kernel_spmd(nc, in_maps, list(range(NCORES))).results
    raw = np.concatenate([res[k]["out"] for k in range(NCORES)], axis=0)
    # device layout per row: [C, NPAIR] even outputs (class-major) + one
    # [NPAIR] odd base (odd outputs are class-independent); assemble
    # out[b, 2j + parity, c] with the odd base broadcast across classes
    HALF = C * NPAIR
    ev = raw[:, 0:HALF].astype(np.float32).reshape(B, C, NPAIR)
    od = raw[:, HALF: HALF + NPAIR].astype(np.float32)
    out = np.empty((B, NPAIR, 2, C), dtype=np.float32)
    out[:, :, 0, :] = ev.transpose(0, 2, 1)
    out[:, :, 1, :] = od[:, :, None]
    return out.reshape(B, NG, C)
